# revision 5
# baseline (speedup 1.0000x reference)
"""BinaryConv2d (3x3, SAME, NHWC) Trainium2 Bass kernel.

Strategy:
  - Data-parallel over batch: 32 images -> 8 cores x 4 images. Weights/bias
    replicated. No collectives needed.
  - Host prep (tiny): Wq = sign(W) cast to bf16 (+-1 exact), laid out as
    [cin, 9, cout]; bias replicated to [128, cout] f32.
  - Per core, per image:
      1. SWDGE cast-DMA: x[img] f32 NHWC (HBM) -> bf16 [H, W+2, cin] HBM
         scratch, middle columns; columns 0 and W+1 are zeroed by two small
         DMAs from a zero SBUF tile (left/right SAME pads).
      2. HWDGE xbar transpose-DMA: scratch [(H*(W+2)), cin] -> SBUF
         xT [cin, H*(W+2)] contiguous, channel-major.
      3. For each output row r: accumulate 9 (clipped at top/bottom) matmuls
         into PSUM [W, cout]: lhsT = xT[:, (r+dh-1)*(W+2)+dw : +W]
         (stationary, pixels on PSUM partitions), rhs = Wq[:, 3*dh+dw, :]
         (streaming, cout free dim). fp32 PSUM accumulation.
      4. DVE tensor_add(psum, bias) -> SBUF f32, HWDGE DMA out to NHWC HBM.
"""

import numpy as np

N_CORES = 8
H = 112
W_DIM = 112
CIN = 128
COUT = 256
BATCH = 32
IMG_PER_CORE = BATCH // N_CORES


def _build_program(n_img, h, w, cin, cout):
    import concourse.bacc as bacc
    import concourse.mybir as mybir
    import concourse.tile as tile

    f32 = mybir.dt.float32
    bf16 = mybir.dt.bfloat16

    nc = bacc.Bacc(
        "TRN2", target_bir_lowering=False, debug=False, num_devices=N_CORES
    )
    x_d = nc.dram_tensor("x", [n_img, h, w, cin], f32, kind="ExternalInput").ap()
    w_d = nc.dram_tensor("w", [cin, 9, cout], bf16, kind="ExternalInput").ap()
    b_d = nc.dram_tensor("b", [128, cout], f32, kind="ExternalInput").ap()
    out_d = nc.dram_tensor(
        "out", [n_img, h, w, cout], f32, kind="ExternalOutput"
    ).ap()

    wp = w + 2  # padded row width in the transposed SBUF image
    rc = 16  # rows per cast/transpose chunk; (rc * wp) % 16 == 0 required
    assert h % rc == 0 and (rc * wp) % 16 == 0
    n_chunks = h // rc

    with tile.TileContext(nc) as tc:
        with (
            tc.tile_pool(name="consts", bufs=1) as cpool,
            tc.tile_pool(name="scratch", bufs=4, space="DRAM") as dpool,
            tc.tile_pool(name="xt", bufs=6) as xtpool,
            tc.tile_pool(name="psum", bufs=8, space="PSUM") as pspool,
            tc.tile_pool(name="outs", bufs=4) as opool,
        ):
            w_t = cpool.tile([cin, 9, cout], bf16)
            nc.sync.dma_start(out=w_t[:], in_=w_d[:])
            b_t = cpool.tile([128, cout], f32)
            nc.sync.dma_start(out=b_t[:], in_=b_d[:])
            zt = cpool.tile([rc, cin], bf16)
            nc.vector.memset(zt[:], 0.0)

            # per-image list of transposed 16-row chunk tiles, built lazily
            # ahead of the consuming rows; chunk tiles: [cin, rc*wp] bf16,
            # input row i of image img lives in chunks[img][i // rc] at col
            # (i % rc) * wp (+1 for the left pad).
            chunks = [[None] * n_chunks for _ in range(n_img)]

            def make_chunk(img, c):
                scr = dpool.tile([rc, wp, cin], bf16, tag="scr")
                nc.sync.dma_start(out=scr[:, 0, :], in_=zt[:])
                nc.sync.dma_start(out=scr[:, wp - 1, :], in_=zt[:])
                # f32 -> bf16 cast during DMA (SWDGE only)
                nc.gpsimd.dma_start(
                    out=scr[:, 1 : w + 1, :], in_=x_d[img, c * rc : (c + 1) * rc]
                )
                xt = xtpool.tile([cin, rc * wp], bf16, tag="xt")
                nc.sync.dma_start(
                    out=xt[:],
                    in_=scr[:].rearrange("a b c -> (a b) c"),
                    transpose=True,
                )
                chunks[img][c] = xt

            def get_row(img, i):
                # lhsT base AP for input row i of image img
                return chunks[img][i // rc], (i % rc) * wp

            all_chunks = [(g, c) for g in range(n_img) for c in range(n_chunks)]
            next_chunk = 0

            def prefetch_to(global_idx):
                nonlocal next_chunk
                while next_chunk <= min(global_idx, len(all_chunks) - 1):
                    make_chunk(*all_chunks[next_chunk])
                    next_chunk += 1

            prefetch_to(1)

            for img in range(n_img):
                for r in range(h):
                    # keep the chunk pipeline ~2 chunks ahead of the consumer
                    prefetch_to(img * n_chunks + r // rc + 2)

                    ps = pspool.tile([w, cout], f32)
                    taps = [
                        (dh, dw)
                        for dh in (0, 1, 2)
                        for dw in (0, 1, 2)
                        if 0 <= r + dh - 1 < h
                    ]
                    last = len(taps) - 1
                    for k, (dh, dw) in enumerate(taps):
                        xt, base = get_row(img, r + dh - 1)
                        nc.tensor.matmul(
                            ps[:],
                            xt[:, base + dw : base + dw + w],
                            w_t[:, 3 * dh + dw, :],
                            start=(k == 0),
                            stop=(k == last),
                        )
                    ot = opool.tile([w, cout], f32)
                    nc.vector.tensor_add(ot[:], ps[:], b_t[:w, :])
                    nc.scalar.dma_start(out=out_d[img, r], in_=ot[:])

    nc.compile()
    return nc


_cached_nc = None


def _get_program():
    global _cached_nc
    if _cached_nc is None:
        _cached_nc = _build_program(IMG_PER_CORE, H, W_DIM, CIN, COUT)
    return _cached_nc


def _prep_inputs(x, W, b):
    import ml_dtypes

    # sign with sign(0)=0, matching jnp.sign; bf16 holds +-1/0 exactly
    wq = np.sign(W.astype(np.float32)).astype(ml_dtypes.bfloat16)
    # [3,3,cin,cout] -> [cin, 9, cout]
    wq = np.ascontiguousarray(wq.transpose(2, 0, 1, 3).reshape(CIN, 9, COUT))
    b_rep = np.ascontiguousarray(
        np.broadcast_to(b.astype(np.float32), (128, COUT))
    )
    in_maps = []
    for c in range(N_CORES):
        xs = np.ascontiguousarray(
            x[c * IMG_PER_CORE : (c + 1) * IMG_PER_CORE].astype(np.float32)
        )
        in_maps.append({"x": xs, "w": wq, "b": b_rep})
    return in_maps


def run(x, W, b, trace=False, tmpdir=None):
    from concourse import bass_utils

    if trace:
        # the agent image's antenv lacks axon_hooks; wire the NTFF profile
        # hook up manually so trace=True yields exec_time_ns + pftrace
        import sys, types

        if "antenv.axon_hooks" not in sys.modules:
            import antenv
            from trn_agent_boot.trn_boot import _ntff_profile_via_ctypes

            mod = types.ModuleType("antenv.axon_hooks")
            _hook = _ntff_profile_via_ctypes("/opt/axon/libaxon_pjrt.so")
            mod.get_axon_ntff_profile_hook = lambda: _hook
            sys.modules["antenv.axon_hooks"] = mod
            antenv.axon_hooks = mod

    nc = _get_program()
    in_maps = _prep_inputs(x, W, b)
    res = bass_utils.run_bass_kernel_spmd(
        nc, in_maps, list(range(N_CORES)), trace=trace, tmpdir=tmpdir
    )
    out = np.concatenate([res.results[i]["out"] for i in range(N_CORES)], axis=0)
    return out, res


def kernel(x, W, b):
    out, _ = run(x, W, b, trace=False)
    return out
